# revision 1
# baseline (speedup 1.0000x reference)
"""Cross-attention kernel for 8 TRN2 NeuronCores.

Sharding: core c -> batch b = c//2, head-group g = c%2 (8 of 16 heads).
Each core computes its batch's attention for its 8 heads plus the
row-sharded slice of the output projection; the host sums the two
partial outputs per batch and adds bo.  No device collectives.

Layouts (per core):
  xT, encT      [1024, 2048]  bf16  (d_model on partitions)
  wq/wk/wv      [1024, 512]   bf16  (f = (head, e) flattened, head-major)
  wo            [512, 1024]   bf16
  QT, KT        [512, r]      bf16  (f on partitions)   = W.T @ xT (+b)
  V (natural)   [2048, 8, 65] bf16  (s on partitions, col 64 = ones)
  scores^T      psum f32 [s_tile=128, 3, r=512] batches = KT_h.T-chunk @ QT_h
                (two heads of a pair run concurrently on PE row groups 0/64)
  expS          bf16 [128, 32, 512], exp(0.125*scores^T) in N=1536 batches
                (no max subtraction: |scores| < ~3)
  attnV         psum [65, 512] = [V_h | 1].T @ expS  (row 64 = softmax denom)
  MH^T          [512, 512-block] bf16 = attnV[0:64] * bcast(1/denom)
  out^T         [1024, 2048] f32 = wo.T @ MH^T

The QT projection for r-block r+1 is computed inside r-block r's head
loop so the PE has ready gap-filler work while ScalarE (the attention
phase bottleneck) grinds through the exps -- this keeps PE dense and the
HAM clock at 2.4 GHz.
"""
import sys
import numpy as np

try:
    import concourse.bass as bass
except ImportError:
    sys.path.insert(0, "/opt/trn_rl_repo")
    import concourse.bass as bass

import ml_dtypes
from contextlib import ExitStack

import concourse.mybir as mybir
from concourse import bacc
from concourse.tile import TileContext
from concourse.bass_utils import run_bass_kernel_spmd

BF16 = ml_dtypes.bfloat16
FP32 = mybir.dt.float32
BF = mybir.dt.bfloat16

B, S, D = 4, 2048, 1024          # batch, seq (q and kv), d_model
H, E = 8, 64                     # heads per core, head dim
F = H * E                        # 512, per-core projection width
KT = 8                           # k tiles over d_model
FT = 4                           # f tiles over F
ST = 16                          # s tiles over S
RB = 512                         # r block (matmul moving dim)
NRB = S // RB                    # 4
EB = 2                           # score-psum banks per exp batch (N=1024)

_CACHE = {}


def _build(debug=False):
    nc = bacc.Bacc()
    xT = nc.declare_dram_parameter("xT", [D, S], BF, isOutput=False)
    encT = nc.declare_dram_parameter("encT", [D, S], BF, isOutput=False)
    wq = nc.declare_dram_parameter("wq", [D, F], BF, isOutput=False)
    wk = nc.declare_dram_parameter("wk", [D, F], BF, isOutput=False)
    wv = nc.declare_dram_parameter("wv", [D, F], BF, isOutput=False)
    wo = nc.declare_dram_parameter("wo", [F, D], BF, isOutput=False)
    bq = nc.declare_dram_parameter("bq", [F, 1], FP32, isOutput=False)
    bk = nc.declare_dram_parameter("bk", [F, 1], FP32, isOutput=False)
    bvb = nc.declare_dram_parameter("bvb", [1, F], BF, isOutput=False)
    out = nc.declare_dram_parameter("out", [D, S], FP32, isOutput=True)
    if debug:
        dqt = nc.declare_dram_parameter("dqt", [128, FT, RB], BF, isOutput=True)
        dkt = nc.declare_dram_parameter("dkt", [128, FT, S], BF, isOutput=True)
        dv = nc.declare_dram_parameter("dv", [128, ST, H, E + 1], BF, isOutput=True)
        dexp = nc.declare_dram_parameter("dexp", [128, ST * 2, RB], BF, isOutput=True)
        drec = nc.declare_dram_parameter("drec", [128, RB], FP32, isOutput=True)
        dmh = nc.declare_dram_parameter("dmh", [128, FT, RB], BF, isOutput=True)

    Exp = mybir.ActivationFunctionType.Exp
    Copy = mybir.ActivationFunctionType.Copy

    with TileContext(nc) as tc, ExitStack() as ctx:
        wp = ctx.enter_context(tc.tile_pool(name="weights", bufs=1))
        ap = ctx.enter_context(tc.tile_pool(name="acts", bufs=1))
        pp = ctx.enter_context(tc.tile_pool(name="psum", bufs=2, space="PSUM"))

        # ---- weights + biases in (gpsimd SWDGE queues; sync HWDGE is kept
        # for latency-sensitive small DMAs + output) ----
        wq_s = wp.tile([128, KT, F], BF, tag="wq")
        wk_s = wp.tile([128, KT, F], BF, tag="wk")
        wv_s = wp.tile([128, KT, F], BF, tag="wv")
        for k in range(KT):
            nc.gpsimd.dma_start(out=wv_s[:, k, :], in_=wv[k * 128:(k + 1) * 128, :])
            nc.gpsimd.dma_start(out=wk_s[:, k, :], in_=wk[k * 128:(k + 1) * 128, :])
        bq_s = wp.tile([128, FT], FP32, tag="bq")
        bk_s = wp.tile([128, FT], FP32, tag="bk")
        for f in range(FT):
            nc.sync.dma_start(out=bq_s[:, f:f + 1], in_=bq[f * 128:(f + 1) * 128, :])
            nc.sync.dma_start(out=bk_s[:, f:f + 1], in_=bk[f * 128:(f + 1) * 128, :])
        bv_bf = wp.tile([1, F], BF, tag="bvb")
        ones_r = wp.tile([1, 128], BF, tag="ones")
        ones_c = wp.tile([128, 1], BF, tag="onesc")
        nc.vector.memset(ones_r, 1.0)
        nc.vector.memset(ones_c, 1.0)

        atp = ctx.enter_context(tc.tile_pool(name="attn", bufs=1))
        dsp = ctx.enter_context(tc.tile_pool(name="dscratch", bufs=2, space="DRAM"))

        def load_enc(r):
            e_r = atp.tile([128, KT, RB], BF, tag="enc", bufs=2)
            for k in range(KT):
                nc.gpsimd.dma_start(
                    out=e_r[:, k, :],
                    in_=encT[k * 128:(k + 1) * 128, r * RB:(r + 1) * RB])
            return e_r

        def load_x(r):
            x_r = atp.tile([128, KT, RB], BF, tag="xs", bufs=2)
            for k in range(KT):
                nc.gpsimd.dma_start(
                    out=x_r[:, k, :],
                    in_=xT[k * 128:(k + 1) * 128, r * RB:(r + 1) * RB])
            return x_r

        def qt_proj(x_r, f, qt_r):
            ps = pp.tile([128, RB], FP32, tag="ps_o", bufs=4)
            for k in range(KT):
                nc.tensor.matmul(
                    ps, wq_s[:, k, f * 128:(f + 1) * 128], x_r[:, k, :],
                    start=(k == 0), stop=(k == KT - 1))
            nc.vector.tensor_scalar_add(out=qt_r[:, f, :], in0=ps,
                                        scalar1=bq_s[:, f:f + 1])

        nc.sync.dma_start(out=bv_bf, in_=bvb[:])

        # ---- V projection first (feeds attnV; ScalarE does the psum->V
        # copies so it has warm-up work while PE runs dense matmuls) ----
        kt_s = ap.tile([128, FT, S], BF, tag="kts")
        qt_cur = atp.tile([128, FT, RB], BF, tag="qt", bufs=2)
        v_s = ap.tile([128, ST, H, E + 1], BF, tag="vs")
        nc.vector.memset(v_s[:, :, :, E:E + 1], 1.0)
        for rc in range(NRB):
            e_r = load_enc(rc)
            for sl in range(4):
                s = rc * 4 + sl
                ps = pp.tile([128, RB], FP32, tag="ps_o", bufs=4)
                nc.tensor.matmul(ps, ones_r[:], bv_bf[:],
                                 start=True, stop=False)
                for k in range(KT):
                    nc.tensor.matmul(
                        ps, e_r[:, k, sl * 128:(sl + 1) * 128], wv_s[:, k, :],
                        start=False, stop=(k == KT - 1))
                nc.scalar.activation(
                    out=v_s[:, s, :, 0:E],
                    in_=ps.rearrange("p (h e) -> p h e", h=H),
                    func=Copy, bias=0.0, scale=1.0)
                # KT projection chain for f=sl on the same enc chunk --
                # enc is streamed exactly once for both KT and V
                psk = pp.tile([128, RB], FP32, tag="ps_o", bufs=4)
                for k in range(KT):
                    nc.tensor.matmul(
                        psk, wk_s[:, k, sl * 128:(sl + 1) * 128], e_r[:, k, :],
                        start=(k == 0), stop=(k == KT - 1))
                nc.vector.tensor_scalar_add(
                    out=kt_s[:, sl, rc * RB:(rc + 1) * RB], in0=psk,
                    scalar1=bk_s[:, sl:sl + 1])

        x_cur = load_x(0)
        # remaining weights via the (idle) sync queue so they don't delay
        # the gpsimd-queue enc reloads that r-block 0's KT chunks need
        for k in range(KT):
            nc.sync.dma_start(out=wq_s[:, k, :], in_=wq[k * 128:(k + 1) * 128, :])
        wo_s = wp.tile([128, FT, D], BF, tag="wo")
        for f in range(FT):
            nc.sync.dma_start(out=wo_s[:, f, :], in_=wo[f * 128:(f + 1) * 128, :])

        def out_proj(mh_t, rr):
            orsl = slice(rr * RB, (rr + 1) * RB)
            for dt in range(8):
                pso = pp.tile([128, RB], FP32, tag="ps_o", bufs=4)
                for f in range(FT):
                    nc.tensor.matmul(
                        pso, wo_s[:, f, dt * 128:(dt + 1) * 128], mh_t[:, f, :],
                        start=(f == 0), stop=(f == FT - 1))
                osb = atp.tile([128, RB], FP32, tag="osb", bufs=3)
                nc.vector.tensor_copy(out=osb, in_=pso)
                nc.sync.dma_start(out=out[dt * 128:(dt + 1) * 128, orsl], in_=osb)

        # ---- attention + output projection, per r block ----
        for r in range(NRB):
            rsl = slice(r * RB, (r + 1) * RB)
            mh_s = atp.tile([128, FT, RB], BF, tag="mh", bufs=2)
            if r + 1 < NRB:
                x_next = load_x(r + 1)
                qt_next = atp.tile([128, FT, RB], BF, tag="qt", bufs=2)
            for hp in range(FT):  # heads 2hp (partitions 0-63), 2hp+1 (64-127)
                if r == 0:
                    qt_proj(x_cur, hp, qt_cur)
                expS = atp.tile([128, ST * 2, RB], BF, tag="expS", bufs=2)
                # scores^T + exp, in EB-bank psum batches over the 32
                # (s_tile, head01) slices
                j = 0
                while j < ST * 2:
                    bn = min(EB, ST * 2 - j)
                    psB = pp.tile([128, EB, RB], FP32, tag="ps_sc", bufs=2)
                    for jj in range(bn):
                        s, hh = (j + jj) // 2, (j + jj) % 2
                        ssl = slice(s * 128, (s + 1) * 128)
                        pr = slice(hh * 64, hh * 64 + 64)
                        nc.tensor.matmul(
                            psB[:, jj, :], kt_s[pr, hp, ssl], qt_cur[pr, hp, :],
                            start=True, stop=True, tile_position=(hh * 64, 0))
                    nc.scalar.activation(out=expS[:, j:j + bn, :],
                                         in_=psB[:, 0:bn, :], func=Exp, scale=0.125)
                    j += bn
                if debug and r == 0 and hp == 0:
                    nc.sync.dma_start(out=dexp[:], in_=expS[:])
                # QT projection for the next r block: PE gap-filler while
                # ScalarE works through the exps of this head pair.
                if r + 1 < NRB:
                    qt_proj(x_next, hp, qt_next)
                if hp == 0 and r > 0:
                    # previous block's output projection, deferred so the
                    # next scores reach ScalarE before PE burns these slots
                    out_proj(mh_prev, prev_r)
                for hh in range(2):
                    h = 2 * hp + hh
                    po = pp.tile([128, RB], FP32, tag="ps_o", bufs=4)
                    for s in range(ST):
                        nc.tensor.matmul(
                            po[0:E + 1, :], v_s[:, s, h, :], expS[:, 2 * s + hh, :],
                            start=(s == 0), stop=(s == ST - 1))
                    rec = atp.tile([128, RB], FP32, tag="rec", bufs=2)
                    dsc = dsp.tile([1, RB], FP32, tag="dsc", bufs=4)
                    nc.vector.tensor_copy(out=rec[E:E + 1, :], in_=po[E:E + 1, :])
                    nc.sync.dma_start(out=dsc[:], in_=rec[E:E + 1, :])
                    nc.sync.dma_start(out=rec[0:64, :],
                                      in_=dsc[:].to_broadcast([64, RB]))
                    nc.vector.reciprocal_approx_fast(out=rec[0:64, :],
                                                     in_=rec[0:64, :])
                    if debug and r == 0 and hp == 0 and hh == 0:
                        nc.sync.dma_start(out=drec[:], in_=rec[:])
                    if hh == 0:
                        nc.vector.tensor_mul(
                            out=mh_s[0:64, hp, :], in0=po[0:E, :], in1=rec[0:64, :])
                    else:
                        mtmp = atp.tile([64, RB], BF, tag="mtmp", bufs=2)
                        nc.vector.tensor_mul(out=mtmp, in0=po[0:E, :],
                                             in1=rec[0:64, :])
                        nc.sync.dma_start(out=mh_s[64:128, hp, :], in_=mtmp)
            if debug and r == 0:
                nc.sync.dma_start(out=dmh[:], in_=mh_s[:])
            if r + 1 < NRB:
                x_cur, qt_cur = x_next, qt_next
            mh_prev, prev_r = mh_s, r

        out_proj(mh_prev, prev_r)

        if debug:
            nc.sync.dma_start(out=dv[:], in_=v_s[:])

    nc.finalize()
    return nc


def _prep_in_maps(x, enc, Wq, bq, Wk, bk, Wv, bv, Wo):
    def bfc(a):
        return np.ascontiguousarray(a.astype(BF16))

    in_maps = []
    for c in range(8):
        b, g = c // 2, c % 2
        hs = slice(8 * g, 8 * g + 8)
        in_maps.append({
            "xT": bfc(x[b].T),
            "encT": bfc(enc[b].T),
            "wq": bfc(np.transpose(Wq[hs], (1, 0, 2)).reshape(D, F)),
            "wk": bfc(np.transpose(Wk[hs], (1, 0, 2)).reshape(D, F)),
            "wv": bfc(np.transpose(Wv[hs], (1, 0, 2)).reshape(D, F)),
            "wo": bfc(Wo[F * g:F * (g + 1)]),
            "bq": np.ascontiguousarray(bq[hs].reshape(F, 1), dtype=np.float32),
            "bk": np.ascontiguousarray(bk[hs].reshape(F, 1), dtype=np.float32),
            "bvb": bfc(bv[hs].reshape(1, F)),
        })
    return in_maps


def run(inputs, trace=False, debug=False):
    x = np.asarray(inputs["x"], np.float32)
    enc = np.asarray(inputs["encoder_output"], np.float32)
    Wq = np.asarray(inputs["Wq"], np.float32)
    Wk = np.asarray(inputs["Wk"], np.float32)
    Wv = np.asarray(inputs["Wv"], np.float32)
    Wo = np.asarray(inputs["Wo"], np.float32)
    bq = np.asarray(inputs["bq"], np.float32)
    bk = np.asarray(inputs["bk"], np.float32)
    bv = np.asarray(inputs["bv"], np.float32)
    bo = np.asarray(inputs["bo"], np.float32)

    key = "nc_dbg" if debug else "nc"
    if key not in _CACHE:
        _CACHE[key] = _build(debug=debug)
    nc = _CACHE[key]
    in_maps = _prep_in_maps(x, enc, Wq, bq, Wk, bk, Wv, bv, Wo)
    res = run_bass_kernel_spmd(nc, in_maps, core_ids=list(range(8)), trace=trace)
    out = np.zeros((B, S, D), np.float32)
    for b in range(B):
        out[b] = (np.asarray(res.results[2 * b]["out"], np.float32)
                  + np.asarray(res.results[2 * b + 1]["out"], np.float32)).T + bo
    return out, res


def kernel(**inputs):
    out, _ = run(inputs, trace=False)
    return out



# revision 14
# speedup vs baseline: 1.0913x; 1.0913x over previous
"""Cross-attention kernel for 8 TRN2 NeuronCores.

Sharding: core c -> batch b = c//2, head-group g = c%2 (8 of 16 heads).
Each core computes its batch's attention for its 8 heads plus the
row-sharded slice of the output projection; the host sums the two
partial outputs per batch and adds bo (+ bv @ Wo: since softmax rows
sum to 1, attn @ (V + bv) == attn @ V + bv, so the V bias is folded
into the host-side constant add and the device never touches bv).

Layouts (per core):
  xT, encT      [128, 8, 2048] bf16  (d_model as (p,k), single-DMA loads)
  wq/wk/wv      [128, 8, 512]  bf16  (f = (head, e) flattened, head-major)
  wo            [128, 4, 1024] bf16
  QT, KT        [512, r]       bf16  (f on partitions)   = W.T @ xT (+b)
  V (natural)   [2048, 8, 65]  bf16  (s on partitions, col 64 = ones)
  scores^T      psum f32 [s_tile=128, 2, r=512] batches = KT_h.T-chunk @ QT_h
                (two heads of a pair run concurrently on PE row groups 0/64)
  expS          bf16 [128, 32, 512], exp(0.125*scores^T) in N=1024 batches
                (no max subtraction: |scores*0.125| < ~3).  Odd s-tiles are
                computed on GpSimd via the Schraudolph bit-trick
                (i16 = round(scores*a + b) reinterpreted as bf16) so the two
                engines drain the score psum in parallel; ScalarE alone is
                otherwise the pipeline pacer.
  attnV         psum [65, 512] = [V_h | 1].T @ expS  (row 64 = softmax denom)
  MH^T          [512, 512-block] bf16 = attnV[0:64] * bcast(1/denom)
  out^T         [1024, 2048] f32 = wo.T @ MH^T

Schedule: phase 1 streams enc once, computing V-proj for all s plus the
f=0 slice of K-proj per r-chunk, and issues the (r0, hp0) score/exp
sweep per chunk so the exp engines spin up ~15us into the kernel.  The
attention loop is software-pipelined one stage ahead: iteration i
issues scores+exp for stage i+1 (interleaved with the K-proj f=hp+1
fillers during the r0 block, enc re-streamed across both DMA queues)
and finishes with attnV/normalize for stage i, so ScalarE/GpSimd never
wait on issue order.  Output projection for block r runs as filler
inside block r+1.
"""
import sys
import numpy as np

try:
    import concourse.bass as bass
except ImportError:
    sys.path.insert(0, "/opt/trn_rl_repo")
    import concourse.bass as bass

import ml_dtypes
from contextlib import ExitStack

import concourse.mybir as mybir
from concourse import bacc
from concourse.tile import TileContext
from concourse.bass_utils import run_bass_kernel_spmd

BF16 = ml_dtypes.bfloat16
FP32 = mybir.dt.float32
I16 = mybir.dt.int16
BF = mybir.dt.bfloat16

B, S, D = 4, 2048, 1024          # batch, seq (q and kv), d_model
H, E = 8, 64                     # heads per core, head dim
F = H * E                        # 512, per-core projection width
KT = 8                           # k tiles over d_model
FT = 4                           # f tiles over F
ST = 16                          # s tiles over S
RB = 512                         # r block (matmul moving dim)
NRB = S // RB                    # 4
EB = 2                           # score-psum banks per exp batch (N=1024)

# s-tiles whose exp runs on DVE via the bit-trick (rest: exact, ScalarE).
# GpSimd cannot read PSUM, so DVE is the only engine that can share the
# score-drain work with ScalarE.
DVE_TILES = frozenset((2, 5, 8, 11, 14))
TRICK_A = float(0.125 * np.log2(np.e) * 128.0)
TRICK_B = float(16256.0 - 0.058 * 128.0)

_CACHE = {}


def _build():
    nc = bacc.Bacc()
    xT = nc.declare_dram_parameter("xT", [128, KT, S], BF, isOutput=False)
    encT = nc.declare_dram_parameter("encT", [128, KT, S], BF, isOutput=False)
    wq = nc.declare_dram_parameter("wq", [128, KT, F], BF, isOutput=False)
    wk = nc.declare_dram_parameter("wk", [128, KT, F], BF, isOutput=False)
    wv = nc.declare_dram_parameter("wv", [128, KT, F], BF, isOutput=False)
    wo = nc.declare_dram_parameter("wo", [128, FT, D], BF, isOutput=False)
    bq = nc.declare_dram_parameter("bq", [128, FT], FP32, isOutput=False)
    bk = nc.declare_dram_parameter("bk", [128, FT], FP32, isOutput=False)
    out = nc.declare_dram_parameter("out", [D, S], FP32, isOutput=True)

    Exp = mybir.ActivationFunctionType.Exp
    Copy = mybir.ActivationFunctionType.Copy
    Mult = mybir.AluOpType.mult
    Add = mybir.AluOpType.add

    with TileContext(nc) as tc, ExitStack() as ctx:
        wp = ctx.enter_context(tc.tile_pool(name="weights", bufs=1))
        ap = ctx.enter_context(tc.tile_pool(name="acts", bufs=1))
        pp = ctx.enter_context(tc.tile_pool(name="psum", bufs=2, space="PSUM"))
        atp = ctx.enter_context(tc.tile_pool(name="attn", bufs=1))
        dsp = ctx.enter_context(tc.tile_pool(name="dscratch", bufs=2, space="DRAM"))

        # ---- weight/bias DMAs.  gpsimd (SWDGE) and sync (HWDGE) split the
        # phase-1 critical path (enc even/odd chunks); sync also carries
        # wq/x0 (needed by the first Q projection), biases, and later the
        # odd enc re-streams for the K-proj fillers plus all small
        # outbound traffic. ----
        wv_s = wp.tile([128, KT, F], BF, tag="wv")
        wk_s = wp.tile([128, KT, F], BF, tag="wk")
        wq_s = wp.tile([128, KT, F], BF, tag="wq")
        nc.gpsimd.dma_start(out=wv_s, in_=wv[:])
        nc.gpsimd.dma_start(out=wk_s, in_=wk[:])
        nc.sync.dma_start(out=wq_s, in_=wq[:])
        x_cur = atp.tile([128, KT, RB], BF, tag="xs", bufs=2)
        nc.sync.dma_start(out=x_cur, in_=xT[:, :, 0:RB])
        bq_s = wp.tile([128, FT], FP32, tag="bq")
        bk_s = wp.tile([128, FT], FP32, tag="bk")
        nc.sync.dma_start(out=bq_s, in_=bq[:])
        nc.sync.dma_start(out=bk_s, in_=bk[:])
        wo_s = wp.tile([128, FT, D], BF, tag="wo")

        kt_s = ap.tile([128, FT, S], BF, tag="kts")
        v_s = ap.tile([128, ST, H, E + 1], BF, tag="vs")
        nc.vector.memset(v_s[:, :, :, E:E + 1], 1.0)
        qt_cur = atp.tile([128, FT, RB], BF, tag="qt", bufs=2)

        def load_enc(rc):
            e_r = atp.tile([128, KT, RB], BF, tag="enc", bufs=3)
            eng = nc.gpsimd if rc % 2 == 0 else nc.sync
            eng.dma_start(out=e_r, in_=encT[:, :, rc * RB:(rc + 1) * RB])
            return e_r

        def load_x(r):
            x_r = atp.tile([128, KT, RB], BF, tag="xs", bufs=2)
            nc.sync.dma_start(out=x_r, in_=xT[:, :, r * RB:(r + 1) * RB])
            return x_r

        def qt_proj(x_r, f, qt_r):
            ps = pp.tile([128, RB], FP32, tag="ps_o", bufs=4)
            for k in range(KT):
                nc.tensor.matmul(
                    ps, wq_s[:, k, f * 128:(f + 1) * 128], x_r[:, k, :],
                    start=(k == 0), stop=(k == KT - 1))
            nc.vector.tensor_scalar_add(out=qt_r[:, f, :], in0=ps,
                                        scalar1=bq_s[:, f:f + 1])

        def kt_proj(e_r, f, rc):
            psk = pp.tile([128, RB], FP32, tag="ps_o", bufs=4)
            for k in range(KT):
                nc.tensor.matmul(
                    psk, wk_s[:, k, f * 128:(f + 1) * 128], e_r[:, k, :],
                    start=(k == 0), stop=(k == KT - 1))
            nc.vector.tensor_scalar_add(
                out=kt_s[:, f, rc * RB:(rc + 1) * RB], in0=psk,
                scalar1=bk_s[:, f:f + 1])

        def score_exp_batch(expS, hp, qt_r, j, bn):
            """Scores + exp for batches j..j+bn-1 (j = 2*s_tile + hh)."""
            psB = pp.tile([128, EB, RB], FP32, tag="ps_sc", bufs=2)
            for jj in range(bn):
                s, hh = (j + jj) // 2, (j + jj) % 2
                ssl = slice(s * 128, (s + 1) * 128)
                pr = slice(hh * 64, hh * 64 + 64)
                nc.tensor.matmul(
                    psB[:, jj, :], kt_s[pr, hp, ssl], qt_r[pr, hp, :],
                    start=True, stop=True, tile_position=(hh * 64, 0))
            if (j // 2) in DVE_TILES:
                nc.vector.tensor_scalar(
                    out=expS[:, j:j + bn, :].bitcast(I16),
                    in0=psB[:, 0:bn, :], scalar1=TRICK_A, scalar2=TRICK_B,
                    op0=Mult, op1=Add)
            else:
                nc.scalar.activation(out=expS[:, j:j + bn, :],
                                     in_=psB[:, 0:bn, :], func=Exp, scale=0.125)

        # ---- phase 1: stream enc once -> V proj (all s) + K proj (f=0);
        # interleave the (r0, hp0) score/exp sweep per r-chunk so the exp
        # engines spin up while the PE is still on projections. ----
        expS_cur = atp.tile([128, ST * 2, RB], BF, tag="expS", bufs=2)
        encq = [load_enc(0), load_enc(1), load_enc(2)]
        for rc in range(NRB):
            e_r = encq.pop(0)
            if rc == 0:
                encq.append(load_enc(3))
            kt_proj(e_r, 0, rc)
            for sl in range(4):
                s = rc * 4 + sl
                ps = pp.tile([128, RB], FP32, tag="ps_o", bufs=4)
                for k in range(KT):
                    nc.tensor.matmul(
                        ps, e_r[:, k, sl * 128:(sl + 1) * 128], wv_s[:, k, :],
                        start=(k == 0), stop=(k == KT - 1))
                nc.scalar.activation(
                    out=v_s[:, s, :, 0:E],
                    in_=ps.rearrange("p (h e) -> p h e", h=H),
                    func=Copy, bias=0.0, scale=1.0)
            if rc == 0:
                qt_proj(x_cur, 0, qt_cur)
            for sl in range(4):
                score_exp_batch(expS_cur, 0, qt_cur, (rc * 4 + sl) * 2, EB)
        nc.gpsimd.dma_start(out=wo_s, in_=wo[:])

        def out_proj(mh_t, rr):
            orsl = slice(rr * RB, (rr + 1) * RB)
            for dt in range(8):
                pso = pp.tile([128, RB], FP32, tag="ps_o", bufs=4)
                for f in range(FT):
                    nc.tensor.matmul(
                        pso, wo_s[:, f, dt * 128:(dt + 1) * 128], mh_t[:, f, :],
                        start=(f == 0), stop=(f == FT - 1))
                osb = atp.tile([128, RB], FP32, tag="osb", bufs=3)
                nc.vector.tensor_copy(out=osb, in_=pso)
                nc.sync.dma_start(out=out[dt * 128:(dt + 1) * 128, orsl], in_=osb)

        # ---- attention + output projection, software-pipelined: stage
        # i = (r, hp) issues scores/exp for stage i+1 and consumes the
        # expS of stage i with attnV. ----
        for r in range(NRB):
            mh_s = atp.tile([128, FT, RB], BF, tag="mh", bufs=2)
            if r + 1 < NRB:
                x_next = load_x(r + 1)
                qt_next = atp.tile([128, FT, RB], BF, tag="qt", bufs=2)
            for hp in range(FT):  # heads 2hp (partitions 0-63), 2hp+1 (64-127)
                i = 4 * r + hp
                # -- issue scores + exp for stage i+1 --
                if i + 1 < 4 * NRB:
                    nr, nhp = divmod(i + 1, 4)
                    nqt = qt_cur if nr == r else qt_next
                    expS_next = atp.tile([128, ST * 2, RB], BF, tag="expS",
                                         bufs=2)
                    if r == 0 and nr == 0:
                        # qt slice for (0, nhp) is computed here, just in
                        # time; kt f=nhp comes from the enc re-stream
                        # fillers, interleaved per chunk with the scores
                        # that consume it
                        qt_proj(x_cur, nhp, qt_cur)
                        e2q = [load_enc(0), load_enc(1), load_enc(2)]
                        for rc in range(NRB):
                            e2 = e2q.pop(0)
                            if rc == 0:
                                e2q.append(load_enc(3))
                            kt_proj(e2, nhp, rc)
                            for sl in range(4):
                                score_exp_batch(
                                    expS_next, nhp, nqt, (rc * 4 + sl) * 2, EB)
                    else:
                        j = 0
                        while j < ST * 2:
                            bn = min(EB, ST * 2 - j)
                            score_exp_batch(expS_next, nhp, nqt, j, bn)
                            j += bn
                else:
                    expS_next = None
                # -- PE gap-filler while the exp engines drain stage i+1:
                # next r-block's QT projection, then the deferred output
                # projection of block r-1 --
                if r + 1 < NRB:
                    qt_proj(x_next, hp, qt_next)
                if hp == 0 and r > 0:
                    out_proj(mh_prev, prev_r)
                # -- attnV + normalize for stage i --
                for hh in range(2):
                    h = 2 * hp + hh
                    po = pp.tile([128, RB], FP32, tag="ps_o", bufs=4)
                    for s in range(ST):
                        nc.tensor.matmul(
                            po[0:E + 1, :], v_s[:, s, h, :],
                            expS_cur[:, 2 * s + hh, :],
                            start=(s == 0), stop=(s == ST - 1))
                    rec = atp.tile([128, RB], FP32, tag="rec", bufs=2)
                    dsc = dsp.tile([1, RB], FP32, tag="dsc", bufs=4)
                    nc.vector.tensor_copy(out=rec[E:E + 1, :], in_=po[E:E + 1, :])
                    nc.sync.dma_start(out=dsc[:], in_=rec[E:E + 1, :])
                    nc.sync.dma_start(out=rec[0:64, :],
                                      in_=dsc[:].to_broadcast([64, RB]))
                    nc.vector.reciprocal_approx_fast(out=rec[0:64, :],
                                                     in_=rec[0:64, :])
                    if hh == 0:
                        nc.vector.tensor_mul(
                            out=mh_s[0:64, hp, :], in0=po[0:E, :], in1=rec[0:64, :])
                    else:
                        mtmp = atp.tile([64, RB], BF, tag="mtmp", bufs=2)
                        nc.vector.tensor_mul(out=mtmp, in0=po[0:E, :],
                                             in1=rec[0:64, :])
                        nc.sync.dma_start(out=mh_s[64:128, hp, :], in_=mtmp)
                expS_cur = expS_next
            if r + 1 < NRB:
                x_cur, qt_cur = x_next, qt_next
            mh_prev, prev_r = mh_s, r

        out_proj(mh_prev, prev_r)

    nc.finalize()
    return nc


def _prep_in_maps(x, enc, Wq, bq, Wk, bk, Wv, bv, Wo):
    def kp(a):  # [D, n] -> [128, KT, n]
        return np.ascontiguousarray(
            a.reshape(KT, 128, a.shape[1]).transpose(1, 0, 2).astype(BF16))

    in_maps = []
    for c in range(8):
        b, g = c // 2, c % 2
        hs = slice(8 * g, 8 * g + 8)
        in_maps.append({
            "xT": kp(x[b].T),
            "encT": kp(enc[b].T),
            "wq": kp(np.transpose(Wq[hs], (1, 0, 2)).reshape(D, F)),
            "wk": kp(np.transpose(Wk[hs], (1, 0, 2)).reshape(D, F)),
            "wv": kp(np.transpose(Wv[hs], (1, 0, 2)).reshape(D, F)),
            "wo": np.ascontiguousarray(
                Wo[F * g:F * (g + 1)].reshape(FT, 128, D).transpose(1, 0, 2)
                .astype(BF16)),
            "bq": np.ascontiguousarray(
                bq[hs].reshape(FT, 128).T, dtype=np.float32),
            "bk": np.ascontiguousarray(
                bk[hs].reshape(FT, 128).T, dtype=np.float32),
        })
    return in_maps


def run(inputs, trace=False):
    x = np.asarray(inputs["x"], np.float32)
    enc = np.asarray(inputs["encoder_output"], np.float32)
    Wq = np.asarray(inputs["Wq"], np.float32)
    Wk = np.asarray(inputs["Wk"], np.float32)
    Wv = np.asarray(inputs["Wv"], np.float32)
    Wo = np.asarray(inputs["Wo"], np.float32)
    bq = np.asarray(inputs["bq"], np.float32)
    bk = np.asarray(inputs["bk"], np.float32)
    bv = np.asarray(inputs["bv"], np.float32)
    bo = np.asarray(inputs["bo"], np.float32)

    if "nc" not in _CACHE:
        _CACHE["nc"] = _build()
    nc = _CACHE["nc"]
    in_maps = _prep_in_maps(x, enc, Wq, bq, Wk, bk, Wv, bv, Wo)
    res = run_bass_kernel_spmd(nc, in_maps, core_ids=list(range(8)), trace=trace)
    # host-side: sum the two head-group partials per batch, add bo and the
    # folded V bias (attn rows sum to 1 -> heads bias == bv, so its out-proj
    # contribution is the constant vector bv_flat @ Wo)
    const = bo + bv.reshape(-1) @ Wo
    out = np.zeros((B, S, D), np.float32)
    for b in range(B):
        out[b] = (np.asarray(res.results[2 * b]["out"], np.float32)
                  + np.asarray(res.results[2 * b + 1]["out"], np.float32)).T + const
    return out, res


def kernel(**inputs):
    out, _ = run(inputs, trace=False)
    return out


# revision 23
# speedup vs baseline: 1.1063x; 1.0137x over previous
"""Cross-attention kernel for 8 TRN2 NeuronCores.

Sharding: core c -> batch b = c//2, head-group g = c%2 (8 of 16 heads).
Each core computes its batch's attention for its 8 heads plus the
row-sharded slice of the output projection; the host sums the two
partial outputs per batch and adds bo (+ bv @ Wo: since softmax rows
sum to 1, attn @ (V + bv) == attn @ V + bv, so the V bias is folded
into the host-side constant add and the device never touches bv).

Layouts (per core):
  xT, encT      [128, 8, 2048] bf16  (d_model as (p,k), single-DMA loads)
  wq/wk/wv      [128, 8, 512]  bf16  (f = (head, e) flattened, head-major)
  wo            [128, 4, 1024] bf16
  QT, KT        [512, r]       bf16  (f on partitions)   = W.T @ xT (+b)
  V (natural)   [2048, 8, 65]  bf16  (s on partitions, col 64 = ones)
  scores^T      psum f32 [s_tile=128, 2, r=512] batches = KT_h.T-chunk @ QT_h
                (two heads of a pair run concurrently on PE row groups 0/64)
  expS          bf16 [128, 32, 512], exp(0.125*scores^T) in N=1024 batches
                (no max subtraction: |scores*0.125| < ~3).  Odd s-tiles are
                computed on GpSimd via the Schraudolph bit-trick
                (i16 = round(scores*a + b) reinterpreted as bf16) so the two
                engines drain the score psum in parallel; ScalarE alone is
                otherwise the pipeline pacer.
  attnV         psum [65, 512] = [V_h | 1].T @ expS  (row 64 = softmax denom)
  MH^T          [512, 512-block] bf16 = attnV[0:64] * bcast(1/denom)
  out^T         [1024, 2048] f32 = wo.T @ MH^T

Schedule: phase 1 streams enc once, computing V-proj for all s plus the
f=0 slice of K-proj per r-chunk, and issues the (r0, hp0) score/exp
sweep per chunk so the exp engines spin up ~15us into the kernel.  The
attention loop is software-pipelined one stage ahead: iteration i
issues scores+exp for stage i+1 (interleaved with the K-proj f=hp+1
fillers during the r0 block, enc re-streamed across both DMA queues)
and finishes with attnV/normalize for stage i, so ScalarE/GpSimd never
wait on issue order.  Output projection for block r runs as filler
inside block r+1.
"""
import sys
import numpy as np

try:
    import concourse.bass as bass
except ImportError:
    sys.path.insert(0, "/opt/trn_rl_repo")
    import concourse.bass as bass

import ml_dtypes
from contextlib import ExitStack

import concourse.mybir as mybir
from concourse import bacc
from concourse.tile import TileContext
from concourse.bass_utils import run_bass_kernel_spmd

BF16 = ml_dtypes.bfloat16
FP32 = mybir.dt.float32
I16 = mybir.dt.int16
BF = mybir.dt.bfloat16

B, S, D = 4, 2048, 1024          # batch, seq (q and kv), d_model
H, E = 8, 64                     # heads per core, head dim
F = H * E                        # 512, per-core projection width
KT = 8                           # k tiles over d_model
FT = 4                           # f tiles over F
ST = 16                          # s tiles over S
RB = 512                         # r block (matmul moving dim)
NRB = S // RB                    # 4
EB = 2                           # score-psum banks per exp batch (N=1024)

# s-tiles whose exp runs on DVE via the bit-trick (rest: exact, ScalarE).
# GpSimd cannot read PSUM, so DVE is the only engine that can share the
# score-drain work with ScalarE.
DVE_TILES = frozenset((2, 4, 7, 9, 12, 14))
TRICK_A = float(0.125 * np.log2(np.e) * 128.0)
TRICK_B = float(16256.0 - 0.058 * 128.0)

_CACHE = {}


def _build():
    nc = bacc.Bacc()
    xT = nc.declare_dram_parameter("xT", [128, KT, S], BF, isOutput=False)
    encT = nc.declare_dram_parameter("encT", [128, KT, S], BF, isOutput=False)
    wq = nc.declare_dram_parameter("wq", [128, KT, F], BF, isOutput=False)
    wk = nc.declare_dram_parameter("wk", [128, KT, F], BF, isOutput=False)
    wv = nc.declare_dram_parameter("wv", [128, KT, F], BF, isOutput=False)
    wo = nc.declare_dram_parameter("wo", [128, FT, D], BF, isOutput=False)
    bq = nc.declare_dram_parameter("bq", [128, FT], FP32, isOutput=False)
    bk = nc.declare_dram_parameter("bk", [128, FT], FP32, isOutput=False)
    out = nc.declare_dram_parameter("out", [D, S], FP32, isOutput=True)

    Exp = mybir.ActivationFunctionType.Exp
    Copy = mybir.ActivationFunctionType.Copy
    Mult = mybir.AluOpType.mult
    Add = mybir.AluOpType.add

    with TileContext(nc) as tc, ExitStack() as ctx:
        wp = ctx.enter_context(tc.tile_pool(name="weights", bufs=1))
        ap = ctx.enter_context(tc.tile_pool(name="acts", bufs=1))
        pp = ctx.enter_context(tc.tile_pool(name="psum", bufs=2, space="PSUM"))
        atp = ctx.enter_context(tc.tile_pool(name="attn", bufs=1))
        dsp = ctx.enter_context(tc.tile_pool(name="dscratch", bufs=2, space="DRAM"))

        # ---- weight/bias DMAs.  gpsimd (SWDGE) and sync (HWDGE) split the
        # phase-1 critical path (enc even/odd chunks); sync also carries
        # wq/x0 (needed by the first Q projection), biases, and later the
        # odd enc re-streams for the K-proj fillers plus all small
        # outbound traffic. ----
        wv_s = wp.tile([128, KT, F], BF, tag="wv")
        wk_s = wp.tile([128, KT, F], BF, tag="wk")
        wq_s = wp.tile([128, KT, F], BF, tag="wq")
        nc.gpsimd.dma_start(out=wv_s, in_=wv[:])
        nc.sync.dma_start(out=wq_s, in_=wq[:])
        x_cur = atp.tile([128, KT, RB], BF, tag="xs", bufs=2)
        nc.sync.dma_start(out=x_cur, in_=xT[:, :, 0:RB])
        bq_s = wp.tile([128, FT], FP32, tag="bq")
        bk_s = wp.tile([128, FT], FP32, tag="bk")
        nc.sync.dma_start(out=bq_s, in_=bq[:])
        nc.sync.dma_start(out=bk_s, in_=bk[:])
        wo_s = wp.tile([128, FT, D], BF, tag="wo")

        kt_s = ap.tile([128, FT, S], BF, tag="kts")
        v_s = ap.tile([128, ST, H, E + 1], BF, tag="vs")
        nc.vector.memset(v_s[:, :, :, E:E + 1], 1.0)
        qt_cur = atp.tile([128, FT, RB], BF, tag="qt", bufs=2)

        def load_enc(rc):
            e_r = atp.tile([128, KT, RB], BF, tag="enc", bufs=3)
            eng = nc.gpsimd if rc % 2 == 0 else nc.sync
            eng.dma_start(out=e_r, in_=encT[:, :, rc * RB:(rc + 1) * RB])
            return e_r

        def load_x(r):
            x_r = atp.tile([128, KT, RB], BF, tag="xs", bufs=2)
            nc.sync.dma_start(out=x_r, in_=xT[:, :, r * RB:(r + 1) * RB])
            return x_r

        def qt_proj(x_r, f, qt_r):
            ps = pp.tile([128, RB], FP32, tag="ps_o", bufs=4)
            for k in range(KT):
                nc.tensor.matmul(
                    ps, wq_s[:, k, f * 128:(f + 1) * 128], x_r[:, k, :],
                    start=(k == 0), stop=(k == KT - 1))
            nc.vector.tensor_scalar_add(out=qt_r[:, f, :], in0=ps,
                                        scalar1=bq_s[:, f:f + 1])

        def kt_proj(e_r, f, rc):
            psk = pp.tile([128, RB], FP32, tag="ps_o", bufs=4)
            for k in range(KT):
                nc.tensor.matmul(
                    psk, wk_s[:, k, f * 128:(f + 1) * 128], e_r[:, k, :],
                    start=(k == 0), stop=(k == KT - 1))
            nc.vector.tensor_scalar_add(
                out=kt_s[:, f, rc * RB:(rc + 1) * RB], in0=psk,
                scalar1=bk_s[:, f:f + 1])

        def score_exp_batch(expS, hp, qt_r, j, bn):
            """Scores + exp for batches j..j+bn-1 (j = 2*s_tile + hh)."""
            psB = pp.tile([128, EB, RB], FP32, tag="ps_sc", bufs=2)
            for jj in range(bn):
                s, hh = (j + jj) // 2, (j + jj) % 2
                ssl = slice(s * 128, (s + 1) * 128)
                pr = slice(hh * 64, hh * 64 + 64)
                nc.tensor.matmul(
                    psB[:, jj, :], kt_s[pr, hp, ssl], qt_r[pr, hp, :],
                    start=True, stop=True, tile_position=(hh * 64, 0))
            if (j // 2) in DVE_TILES:
                nc.vector.tensor_scalar(
                    out=expS[:, j:j + bn, :].bitcast(I16),
                    in0=psB[:, 0:bn, :], scalar1=TRICK_A, scalar2=TRICK_B,
                    op0=Mult, op1=Add)
            else:
                nc.scalar.activation(out=expS[:, j:j + bn, :],
                                     in_=psB[:, 0:bn, :], func=Exp, scale=0.125)

        # ---- phase 1: stream enc once -> V proj (all s) + K proj (f=0);
        # interleave the (r0, hp0) score/exp sweep per r-chunk so the exp
        # engines spin up while the PE is still on projections. ----
        expS_cur = atp.tile([128, ST * 2, RB], BF, tag="expS", bufs=2)
        encq = [load_enc(0)]
        nc.gpsimd.dma_start(out=wk_s, in_=wk[:])
        encq += [load_enc(1), load_enc(2)]
        for rc in range(NRB):
            e_r = encq.pop(0)
            if rc == 0:
                encq.append(load_enc(3))
            for sl in range(4):
                s = rc * 4 + sl
                ps = pp.tile([128, RB], FP32, tag="ps_o", bufs=4)
                for k in range(KT):
                    nc.tensor.matmul(
                        ps, e_r[:, k, sl * 128:(sl + 1) * 128], wv_s[:, k, :],
                        start=(k == 0), stop=(k == KT - 1))
                nc.scalar.activation(
                    out=v_s[:, s, :, 0:E],
                    in_=ps.rearrange("p (h e) -> p h e", h=H),
                    func=Copy, bias=0.0, scale=1.0)
            kt_proj(e_r, 0, rc)
            if rc == 0:
                qt_proj(x_cur, 0, qt_cur)
            for sl in range(4):
                score_exp_batch(expS_cur, 0, qt_cur, (rc * 4 + sl) * 2, EB)
        nc.gpsimd.dma_start(out=wo_s, in_=wo[:])

        def out_chunk(mh_t, rr, dt):
            # output projection chunk (psum cannot source a DMA, so the
            # SBUF hop runs on ScalarE, which has slack; DVE gates the
            # attnV-normalize path)
            orsl = slice(rr * RB, (rr + 1) * RB)
            pso = pp.tile([128, RB], FP32, tag="ps_o", bufs=4)
            for f in range(FT):
                nc.tensor.matmul(
                    pso, wo_s[:, f, dt * 128:(dt + 1) * 128], mh_t[:, f, :],
                    start=(f == 0), stop=(f == FT - 1))
            osb = atp.tile([128, RB], FP32, tag="osb", bufs=3)
            nc.scalar.activation(out=osb, in_=pso, func=Copy, bias=0.0,
                                 scale=1.0)
            nc.sync.dma_start(out=out[dt * 128:(dt + 1) * 128, orsl], in_=osb)

        def attnv_half(pos, h, expS_t, s0):
            # psum tile allocated at chain start (pool alloc order must
            # match PE issue order or a WAR against a later-issued chain
            # deadlocks the in-order engine)
            hh = h % 2
            if s0 == 0:
                pos[hh] = pp.tile([128, RB], FP32, tag="ps_o", bufs=4,
                                  name=f"po{hh}")
            po = pos[hh]
            for s in range(s0, s0 + ST // 2):
                nc.tensor.matmul(
                    po[0:E + 1, :], v_s[:, s, h, :], expS_t[:, 2 * s + hh, :],
                    start=(s == 0), stop=(s == ST - 1))

        def attnv_norm(pos, hp, hh, mh_t):
            po = pos[hh]
            rec = atp.tile([128, RB], FP32, tag="rec", bufs=2)
            dsc = dsp.tile([1, RB], FP32, tag="dsc", bufs=4)
            nc.vector.tensor_copy(out=rec[E:E + 1, :], in_=po[E:E + 1, :])
            nc.sync.dma_start(out=dsc[:], in_=rec[E:E + 1, :])
            nc.sync.dma_start(out=rec[0:64, :],
                              in_=dsc[:].to_broadcast([64, RB]))
            nc.vector.reciprocal_approx_fast(out=rec[0:64, :],
                                             in_=rec[0:64, :])
            if hh == 0:
                nc.vector.tensor_mul(
                    out=mh_t[0:64, hp, :], in0=po[0:E, :], in1=rec[0:64, :])
            else:
                mtmp = atp.tile([64, RB], BF, tag="mtmp", bufs=2)
                nc.vector.tensor_mul(out=mtmp, in0=po[0:E, :],
                                     in1=rec[0:64, :])
                nc.sync.dma_start(out=mh_t[64:128, hp, :], in_=mtmp)

        # ---- attention + output projection, software-pipelined: stage
        # i = (r, hp) issues scores/exp for stage i+1 and consumes the
        # expS of stage i with attnV.  The PE is in-order, so score
        # batches (whose psum banks recycle only as fast as the exp
        # engines drain them) are WOVEN with independent filler chains
        # (attnV halves, QT chains, output-projection chunks) in issue
        # order -- the PE always has retired-dependency work in front of
        # it instead of stalling on a psum WAR. ----
        for r in range(NRB):
            mh_s = atp.tile([128, FT, RB], BF, tag="mh", bufs=2)
            if r + 1 < NRB:
                x_next = load_x(r + 1)
                qt_next = atp.tile([128, FT, RB], BF, tag="qt", bufs=2)
            for hp in range(FT):  # heads 2hp (partitions 0-63), 2hp+1 (64-127)
                i = 4 * r + hp
                expS_me = expS_cur
                # filler thunk stream for this stage (each ~0.9-1.7us PE)
                pos = {}
                fillers = []
                if r + 1 < NRB:
                    fillers.append(lambda: qt_proj(x_next, hp, qt_next))
                fillers += [
                    lambda: attnv_half(pos, 2 * hp, expS_me, 0),
                    lambda: attnv_half(pos, 2 * hp, expS_me, ST // 2),
                    lambda: attnv_norm(pos, hp, 0, mh_s),
                    lambda: attnv_half(pos, 2 * hp + 1, expS_me, 0),
                    lambda: attnv_half(pos, 2 * hp + 1, expS_me, ST // 2),
                    lambda: attnv_norm(pos, hp, 1, mh_s),
                ]
                if hp == 0 and r > 0:
                    mp, pr_ = mh_prev, prev_r
                    for dt in range(8):
                        fillers.append(
                            lambda mp=mp, pr_=pr_, dt=dt: out_chunk(mp, pr_, dt))
                # -- scores + exp for stage i+1, woven with the fillers --
                if i + 1 < 4 * NRB:
                    nr, nhp = divmod(i + 1, 4)
                    nqt = qt_cur if nr == r else qt_next
                    expS_next = atp.tile([128, ST * 2, RB], BF, tag="expS",
                                         bufs=2)
                    if r == 0 and nr == 0:
                        # r0: kt f=nhp comes from the enc re-stream fillers,
                        # interleaved per chunk with the scores that consume
                        # it; the kt chains themselves are the weave filler
                        qt_proj(x_cur, nhp, qt_cur)
                        e2q = [load_enc(0), load_enc(1), load_enc(2)]
                        fi = 0
                        for rc in range(NRB):
                            e2 = e2q.pop(0)
                            if rc == 0:
                                e2q.append(load_enc(3))
                            kt_proj(e2, nhp, rc)
                            for sl in range(4):
                                score_exp_batch(
                                    expS_next, nhp, nqt, (rc * 4 + sl) * 2, EB)
                                if sl % 2 == 1 and fi < len(fillers):
                                    fillers[fi]()
                                    fi += 1
                        while fi < len(fillers):
                            fillers[fi]()
                            fi += 1
                    else:
                        nb = ST * 2 // EB  # 16 score batches
                        fi = 0
                        for b in range(nb):
                            score_exp_batch(expS_next, nhp, nqt, b * EB, EB)
                            if b >= 1:  # keep 2 psB batches primed, then weave
                                take = ((b) * len(fillers)) // (nb - 1) - fi
                                for _ in range(take):
                                    fillers[fi]()
                                    fi += 1
                        while fi < len(fillers):
                            fillers[fi]()
                            fi += 1
                else:
                    expS_next = None
                    for f_ in fillers:
                        f_()
                expS_cur = expS_next
            if r + 1 < NRB:
                x_cur, qt_cur = x_next, qt_next
            mh_prev, prev_r = mh_s, r

        for dt in range(8):
            out_chunk(mh_prev, prev_r, dt)

    nc.finalize()
    return nc


def _prep_in_maps(x, enc, Wq, bq, Wk, bk, Wv, bv, Wo):
    def kp(a):  # [D, n] -> [128, KT, n]
        return np.ascontiguousarray(
            a.reshape(KT, 128, a.shape[1]).transpose(1, 0, 2).astype(BF16))

    in_maps = []
    for c in range(8):
        b, g = c // 2, c % 2
        hs = slice(8 * g, 8 * g + 8)
        in_maps.append({
            "xT": kp(x[b].T),
            "encT": kp(enc[b].T),
            "wq": kp(np.transpose(Wq[hs], (1, 0, 2)).reshape(D, F)),
            "wk": kp(np.transpose(Wk[hs], (1, 0, 2)).reshape(D, F)),
            "wv": kp(np.transpose(Wv[hs], (1, 0, 2)).reshape(D, F)),
            "wo": np.ascontiguousarray(
                Wo[F * g:F * (g + 1)].reshape(FT, 128, D).transpose(1, 0, 2)
                .astype(BF16)),
            "bq": np.ascontiguousarray(
                bq[hs].reshape(FT, 128).T, dtype=np.float32),
            "bk": np.ascontiguousarray(
                bk[hs].reshape(FT, 128).T, dtype=np.float32),
        })
    return in_maps


def run(inputs, trace=False):
    x = np.asarray(inputs["x"], np.float32)
    enc = np.asarray(inputs["encoder_output"], np.float32)
    Wq = np.asarray(inputs["Wq"], np.float32)
    Wk = np.asarray(inputs["Wk"], np.float32)
    Wv = np.asarray(inputs["Wv"], np.float32)
    Wo = np.asarray(inputs["Wo"], np.float32)
    bq = np.asarray(inputs["bq"], np.float32)
    bk = np.asarray(inputs["bk"], np.float32)
    bv = np.asarray(inputs["bv"], np.float32)
    bo = np.asarray(inputs["bo"], np.float32)

    if "nc" not in _CACHE:
        _CACHE["nc"] = _build()
    nc = _CACHE["nc"]
    in_maps = _prep_in_maps(x, enc, Wq, bq, Wk, bk, Wv, bv, Wo)
    res = run_bass_kernel_spmd(nc, in_maps, core_ids=list(range(8)), trace=trace)
    # host-side: sum the two head-group partials per batch, add bo and the
    # folded V bias (attn rows sum to 1 -> heads bias == bv, so its out-proj
    # contribution is the constant vector bv_flat @ Wo)
    const = bo + bv.reshape(-1) @ Wo
    out = np.zeros((B, S, D), np.float32)
    for b in range(B):
        out[b] = (np.asarray(res.results[2 * b]["out"], np.float32)
                  + np.asarray(res.results[2 * b + 1]["out"], np.float32)).T + const
    return out, res


def kernel(**inputs):
    out, _ = run(inputs, trace=False)
    return out
